# revision 7
# baseline (speedup 1.0000x reference)
"""Single-head attention (B=8, S=2048, H=1024, D=64) on 8 TRN2 NeuronCores.

Sharding: data-parallel over the batch dim — core b computes batch element b.

Per-core dataflow (everything kept transposed so the PE contraction dim is
always the partition dim and softmax denominators come out of the attn@v
matmul itself):
  xT[h, s]   = PE-transpose of x tiles                 (f32r)
  qT/kT/vT   = W.T @ xT  (+bias fused on ScalarE)      (f32r, [64, S])
  v_aug      = [v | 1] in [t, d] layout                (f32r, [S, 65])
  scoresT    = kT_tile.T @ qT                          (PSUM, [t=128, s=512])
  attnT      = exp(scoresT/8 + mask_bias)  on ScalarE  (f32r)
  outT_aug   = sum_t v_aug.T @ attnT  (PSUM accum)     ([65, 512]; row 64 = denom)
  out        = transpose(outT_aug) , then out[:, :64] * 1/out[:, 64]
"""

import sys

sys.path.insert(0, "/opt/trn_rl_repo")

import numpy as np

B, S, H, D = 8, 2048, 1024, 64
SB = 512          # s-block (streaming block of queries)
NBLK = S // SB    # 4
NT = S // 128     # 16 t-tiles (and s-tiles)
HC = H // 128     # 8 h-chunks
MASK_BIG = 30000.0


def build_nc():
    import concourse.bacc as bacc
    import concourse.mybir as mybir
    import concourse.tile as tile
    from concourse.masks import make_identity

    dt = mybir.dt
    f32, f32r, i32 = dt.float32, dt.float32r, dt.int32
    AF = mybir.ActivationFunctionType

    nc = bacc.Bacc("TRN2", target_bir_lowering=False, debug=False, num_devices=8)

    X = nc.dram_tensor("x_b", [S, H], f32, kind="ExternalInput")
    MASK = nc.dram_tensor("mask_b", [S], i32, kind="ExternalInput")
    WQ = nc.dram_tensor("Wq", [H, D], f32, kind="ExternalInput")
    BQ = nc.dram_tensor("bq", [D], f32, kind="ExternalInput")
    WK = nc.dram_tensor("Wk", [H, D], f32, kind="ExternalInput")
    BK = nc.dram_tensor("bk", [D], f32, kind="ExternalInput")
    WV = nc.dram_tensor("Wv", [H, D], f32, kind="ExternalInput")
    BV = nc.dram_tensor("bv", [D], f32, kind="ExternalInput")
    OUT = nc.dram_tensor("out_b", [S, D], f32, kind="ExternalOutput")

    with tile.TileContext(nc) as tc:
        with (
            tc.tile_pool(name="const", bufs=1) as cpool,
            tc.tile_pool(name="xs", bufs=5) as xs_pool,
            tc.tile_pool(name="xt", bufs=1) as xt_pool,
            tc.tile_pool(name="qkv", bufs=1) as qkv_pool,
            tc.tile_pool(name="attn", bufs=2) as at_pool,
            tc.tile_pool(name="outs", bufs=2) as o_pool,
            tc.tile_pool(name="ps_big", bufs=4, space="PSUM") as ps_big,
            tc.tile_pool(name="ps_acc", bufs=2, space="PSUM") as ps_acc,
            tc.tile_pool(name="ps_sm", bufs=2, space="PSUM") as ps_sm,
        ):
            # ---- constants ----
            ident_f = cpool.tile([128, 128], f32)
            make_identity(nc, ident_f)
            ident = cpool.tile([128, 128], f32r)
            nc.vector.tensor_copy(ident, ident_f)

            wq = cpool.tile([128, HC, D], f32r)
            wk = cpool.tile([128, HC, D], f32r)
            wv = cpool.tile([128, HC, D], f32r)
            nc.gpsimd.dma_start(out=wq, in_=WQ.ap().rearrange("(c p) m -> p c m", p=128))
            nc.gpsimd.dma_start(out=wk, in_=WK.ap().rearrange("(c p) m -> p c m", p=128))
            nc.gpsimd.dma_start(out=wv, in_=WV.ap().rearrange("(c p) m -> p c m", p=128))

            bias_q = cpool.tile([D, 1], f32)
            bias_k = cpool.tile([D, 1], f32)
            bias_v = cpool.tile([D, 1], f32)
            nc.gpsimd.dma_start(out=bias_q, in_=BQ.ap().rearrange("(p o) -> p o", o=1))
            nc.gpsimd.dma_start(out=bias_k, in_=BK.ap().rearrange("(p o) -> p o", o=1))
            nc.gpsimd.dma_start(out=bias_v, in_=BV.ap().rearrange("(p o) -> p o", o=1))

            # mask -> per-t-chunk additive bias column: (mask - 1) * MASK_BIG
            mask_i = cpool.tile([128, NT], i32)
            nc.gpsimd.dma_start(out=mask_i, in_=MASK.ap().rearrange("(c p) -> p c", p=128))
            mask_f = cpool.tile([128, NT], f32)
            nc.vector.tensor_copy(mask_f, mask_i)
            ones_col = cpool.tile([128, 1], f32)
            nc.vector.memset(ones_col, 1.0)

            mask_bias = cpool.tile([128, NT], f32)
            nc.vector.tensor_scalar(
                out=mask_bias, in0=mask_f,
                scalar1=1.0, scalar2=MASK_BIG,
                op0=mybir.AluOpType.subtract, op1=mybir.AluOpType.mult,
            )

            # ---- stage A: load x, transpose to xT ----
            xt = xt_pool.tile([128, HC, S], f32r)  # xT[h % 128, h // 128, s]
            for jb in range(NBLK):
                xs_tiles = []
                for i4 in range(4):
                    xs = xs_pool.tile([128, H], f32r, tag="xs")
                    st = jb * 4 + i4
                    nc.gpsimd.dma_start(out=xs, in_=X[st * 128:(st + 1) * 128, :])
                    xs_tiles.append(xs)
                for c in range(HC):
                    ps = ps_big.tile([128, SB], f32r, tag="big")
                    for i4 in range(4):
                        nc.tensor.transpose(
                            ps[:, i4 * 128:(i4 + 1) * 128],
                            xs_tiles[i4][:, c * 128:(c + 1) * 128],
                            ident,
                        )
                    dst = xt[:, c, jb * SB:(jb + 1) * SB]
                    if c % 2 == 0:
                        nc.vector.tensor_copy(dst, ps)
                    else:
                        nc.scalar.copy(dst, ps)

            # ---- stage B: projections qT, kT, vT (+bias), build v_aug ----
            qT = qkv_pool.tile([D, S], f32r)
            kT = qkv_pool.tile([D, S], f32r)
            vT = qkv_pool.tile([D, S], f32r)
            v_aug = qkv_pool.tile([128, NT, D + 1], f32r)
            for j in range(NBLK):
                sl = slice(j * SB, (j + 1) * SB)
                for w, bias, dstT in ((wq, bias_q, qT), (wk, bias_k, kT), (wv, bias_v, vT)):
                    ps = ps_big.tile([128, SB], f32, tag="big")
                    for c in range(HC):
                        nc.tensor.matmul(
                            ps[0:D, :], w[:, c, :], xt[:, c, sl],
                            start=(c == 0), stop=(c == HC - 1),
                        )
                    nc.scalar.add(dstT[:, sl], ps[0:D, :], bias)
                # v_aug tiles for the 4 s-tiles of this block
                for i4 in range(4):
                    i = j * 4 + i4
                    ps_t = ps_sm.tile([128, 128], f32r, tag="sm")
                    nc.tensor.transpose(
                        ps_t[:, 0:D], vT[:, i * 128:(i + 1) * 128], ident[0:D, 0:D]
                    )
                    nc.vector.tensor_copy(v_aug[:, i, 0:D], ps_t[:, 0:D])
                    nc.vector.tensor_copy(v_aug[:, i, D:D + 1], ones_col)

            # ---- stage C: attention per s-block ----
            for jb in range(NBLK):
                sl = slice(jb * SB, (jb + 1) * SB)
                at = at_pool.tile([128, NT, SB], f32r, tag="at")
                for i in range(NT):
                    ps = ps_big.tile([128, SB], f32, tag="big")
                    nc.tensor.matmul(
                        ps, kT[:, i * 128:(i + 1) * 128], qT[:, sl],
                        start=True, stop=True,
                    )
                    nc.scalar.activation(
                        out=at[:, i, :], in_=ps, func=AF.Exp,
                        bias=mask_bias[:, i:i + 1], scale=0.125,
                    )
                ps_o = ps_acc.tile([128, SB], f32, tag="acc")
                for i in range(NT):
                    nc.tensor.matmul(
                        ps_o[0:D + 1, :], v_aug[:, i, :], at[:, i, :],
                        start=(i == 0), stop=(i == NT - 1),
                    )
                # transpose partition count must be a multiple of 32 -> pad
                # 65 rows to 96 (pad rows get dup data; their cols are unused)
                o_t = o_pool.tile([96, SB], f32r, tag="ot")
                nc.scalar.copy(o_t[0:D, :], ps_o[0:D, :])
                # row 64 is the denominator; rows 65:95 are stale psum,
                # copied only to keep the 32-aligned transpose input defined
                nc.scalar.copy(o_t[D:96, :], ps_o[D:96, :])
                for st in range(4):
                    ps_t = ps_sm.tile([128, 128], f32r, tag="sm")
                    nc.tensor.transpose(
                        ps_t[:, 0:96],
                        o_t[:, st * 128:(st + 1) * 128],
                        ident[0:96, 0:96],
                    )
                    recip = o_pool.tile([128, 1], f32, tag="recip")
                    nc.vector.reciprocal(recip, ps_t[:, D:D + 1])
                    outf = o_pool.tile([128, D], f32, tag="outf")
                    nc.vector.tensor_scalar_mul(outf, ps_t[:, 0:D], recip)
                    row = (jb * 4 + st) * 128
                    nc.sync.dma_start(out=OUT[row:row + 128, :], in_=outf)

    nc.compile()
    return nc


_NC = None


def kernel(x, mask, Wq, bq, Wk, bk, Wv, bv):
    global _NC
    if _NC is None:
        _NC = build_nc()
    from concourse.bass_utils import run_bass_kernel_spmd

    x = np.ascontiguousarray(np.asarray(x, dtype=np.float32))
    mask = np.ascontiguousarray(np.asarray(mask, dtype=np.int32))
    shared = {
        "Wq": np.asarray(Wq, np.float32), "bq": np.asarray(bq, np.float32),
        "Wk": np.asarray(Wk, np.float32), "bk": np.asarray(bk, np.float32),
        "Wv": np.asarray(Wv, np.float32), "bv": np.asarray(bv, np.float32),
    }
    in_maps = [dict(x_b=x[c], mask_b=mask[c], **shared) for c in range(B)]
    res = run_bass_kernel_spmd(_NC, in_maps, core_ids=list(range(B)))
    return np.stack([res.results[c]["out_b"] for c in range(B)], axis=0)


# revision 10
# speedup vs baseline: 917.3775x; 917.3775x over previous
"""Single-head attention (B=8, S=2048, H=1024, D=64) on 8 TRN2 NeuronCores.

Sharding: data-parallel over the batch dim — core b computes batch element b.

Per-core dataflow (everything kept transposed so the PE contraction dim is
always the partition dim and softmax denominators come out of the attn@v
matmul itself):
  xT[h, s]   = PE-transpose of x tiles                 (f32r)
  qT/kT/vT   = W.T @ xT  (+bias fused on ScalarE)      (f32r, [64, S])
  v_aug      = [v | 1] in [t, d] layout                (f32r, [S, 65])
  scoresT    = kT_tile.T @ qT                          (PSUM, [t=128, s=512])
  attnT      = exp(scoresT/8 + mask_bias)  on ScalarE  (f32r)
  outT_aug   = sum_t v_aug.T @ attnT  (PSUM accum)     ([65, 512]; row 64 = denom)
  out        = transpose(outT_aug) , then out[:, :64] * 1/out[:, 64]
"""

import sys

sys.path.insert(0, "/opt/trn_rl_repo")

import numpy as np

B, S, H, D = 8, 2048, 1024, 64
SB = 512          # s-block (streaming block of queries)
NBLK = S // SB    # 4
NT = S // 128     # 16 t-tiles (and s-tiles)
HC = H // 128     # 8 h-chunks
MASK_BIG = 30000.0


def build_nc(repeats=1):
    import concourse.bacc as bacc
    import concourse.mybir as mybir
    import concourse.tile as tile
    from concourse.masks import make_identity

    dt = mybir.dt
    f32, f32r, i32 = dt.float32, dt.float32r, dt.int32
    AF = mybir.ActivationFunctionType

    nc = bacc.Bacc("TRN2", target_bir_lowering=False, debug=False, num_devices=8)

    X = nc.dram_tensor("x_b", [S, H], f32, kind="ExternalInput")
    MASK = nc.dram_tensor("mask_b", [S], i32, kind="ExternalInput")
    WQ = nc.dram_tensor("Wq", [H, D], f32, kind="ExternalInput")
    BQ = nc.dram_tensor("bq", [D], f32, kind="ExternalInput")
    WK = nc.dram_tensor("Wk", [H, D], f32, kind="ExternalInput")
    BK = nc.dram_tensor("bk", [D], f32, kind="ExternalInput")
    WV = nc.dram_tensor("Wv", [H, D], f32, kind="ExternalInput")
    BV = nc.dram_tensor("bv", [D], f32, kind="ExternalInput")
    OUT = nc.dram_tensor("out_b", [S, D], f32, kind="ExternalOutput")

    with tile.TileContext(nc) as tc:
        with (
            tc.tile_pool(name="const", bufs=1) as cpool,
            tc.tile_pool(name="xs", bufs=5) as xs_pool,
            tc.tile_pool(name="xt", bufs=1) as xt_pool,
            tc.tile_pool(name="qkv", bufs=1) as qkv_pool,
            tc.tile_pool(name="attn", bufs=2) as at_pool,
            tc.tile_pool(name="outs", bufs=2) as o_pool,
            tc.tile_pool(name="ps_big", bufs=4, space="PSUM") as ps_big,
            tc.tile_pool(name="ps_acc", bufs=2, space="PSUM") as ps_acc,
            tc.tile_pool(name="ps_sm", bufs=2, space="PSUM") as ps_sm,
        ):
            # ---- constants ----
            ident_f = cpool.tile([128, 128], f32)
            make_identity(nc, ident_f)
            ident = cpool.tile([128, 128], f32r)
            nc.vector.tensor_copy(ident, ident_f)

            wq = cpool.tile([128, HC, D], f32r)
            wk = cpool.tile([128, HC, D], f32r)
            wv = cpool.tile([128, HC, D], f32r)
            nc.gpsimd.dma_start(out=wq, in_=WQ.ap().rearrange("(c p) m -> p c m", p=128))
            nc.gpsimd.dma_start(out=wk, in_=WK.ap().rearrange("(c p) m -> p c m", p=128))
            nc.gpsimd.dma_start(out=wv, in_=WV.ap().rearrange("(c p) m -> p c m", p=128))

            bias_q = cpool.tile([D, 1], f32)
            bias_k = cpool.tile([D, 1], f32)
            bias_v = cpool.tile([D, 1], f32)
            nc.gpsimd.dma_start(out=bias_q, in_=BQ.ap().rearrange("(p o) -> p o", o=1))
            nc.gpsimd.dma_start(out=bias_k, in_=BK.ap().rearrange("(p o) -> p o", o=1))
            nc.gpsimd.dma_start(out=bias_v, in_=BV.ap().rearrange("(p o) -> p o", o=1))

            # mask -> per-t-chunk additive bias column: (mask - 1) * MASK_BIG
            mask_i = cpool.tile([128, NT], i32)
            nc.gpsimd.dma_start(out=mask_i, in_=MASK.ap().rearrange("(c p) -> p c", p=128))
            mask_f = cpool.tile([128, NT], f32)
            nc.vector.tensor_copy(mask_f, mask_i)
            ones_col = cpool.tile([128, 1], f32)
            nc.vector.memset(ones_col, 1.0)
            mask_bias = cpool.tile([128, NT], f32)
            nc.vector.tensor_scalar(
                out=mask_bias, in0=mask_f,
                scalar1=1.0, scalar2=MASK_BIG,
                op0=mybir.AluOpType.subtract, op1=mybir.AluOpType.mult,
            )

            for _rep in range(repeats):
                # ---- stage A: load x, transpose to xT ----
                xt = xt_pool.tile([128, HC, S], f32r, tag="xt")
                for jb in range(NBLK):
                    xs_tiles = []
                    for i4 in range(4):
                        xs = xs_pool.tile([128, H], f32r, tag="xs")
                        st = jb * 4 + i4
                        nc.gpsimd.dma_start(out=xs, in_=X[st * 128:(st + 1) * 128, :])
                        xs_tiles.append(xs)
                    for c in range(HC):
                        ps = ps_big.tile([128, SB], f32r, tag="big")
                        for i4 in range(4):
                            nc.tensor.transpose(
                                ps[:, i4 * 128:(i4 + 1) * 128],
                                xs_tiles[i4][:, c * 128:(c + 1) * 128],
                                ident,
                            )
                        dst = xt[:, c, jb * SB:(jb + 1) * SB]
                        if c % 2 == 0:
                            nc.vector.tensor_copy(dst, ps)
                        else:
                            nc.scalar.copy(dst, ps)

                # ---- stage B: projections qT, kT, vT (+bias), build v_aug ----
                qT = qkv_pool.tile([D, S], f32r, tag="qT")
                kT = qkv_pool.tile([D, S], f32r, tag="kT")
                vT = qkv_pool.tile([D, S], f32r, tag="vT")
                v_aug = qkv_pool.tile([128, NT, D + 1], f32r, tag="v_aug")
                for j in range(NBLK):
                    sl = slice(j * SB, (j + 1) * SB)
                    for w, bias, dstT in ((wq, bias_q, qT), (wk, bias_k, kT), (wv, bias_v, vT)):
                        ps = ps_big.tile([128, SB], f32, tag="big")
                        for c in range(HC):
                            nc.tensor.matmul(
                                ps[0:D, :], w[:, c, :], xt[:, c, sl],
                                start=(c == 0), stop=(c == HC - 1),
                            )
                        nc.scalar.add(dstT[:, sl], ps[0:D, :], bias)
                    # v_aug tiles for the 4 s-tiles of this block
                    for i4 in range(4):
                        i = j * 4 + i4
                        ps_t = ps_sm.tile([128, 128], f32r, tag="sm")
                        nc.tensor.transpose(
                            ps_t[:, 0:D], vT[:, i * 128:(i + 1) * 128], ident[0:D, 0:D]
                        )
                        nc.vector.tensor_copy(v_aug[:, i, 0:D], ps_t[:, 0:D])
                        nc.vector.tensor_copy(v_aug[:, i, D:D + 1], ones_col)

                # ---- stage C: attention per s-block ----
                for jb in range(NBLK):
                    sl = slice(jb * SB, (jb + 1) * SB)
                    at = at_pool.tile([128, NT, SB], f32r, tag="at")
                    for i in range(NT):
                        ps = ps_big.tile([128, SB], f32, tag="big")
                        nc.tensor.matmul(
                            ps, kT[:, i * 128:(i + 1) * 128], qT[:, sl],
                            start=True, stop=True,
                        )
                        nc.scalar.activation(
                            out=at[:, i, :], in_=ps, func=AF.Exp,
                            bias=mask_bias[:, i:i + 1], scale=0.125,
                        )
                    ps_o = ps_acc.tile([128, SB], f32, tag="acc")
                    for i in range(NT):
                        nc.tensor.matmul(
                            ps_o[0:D + 1, :], v_aug[:, i, :], at[:, i, :],
                            start=(i == 0), stop=(i == NT - 1),
                        )
                    # transpose partition count must be a multiple of 32 -> pad
                    # 65 rows to 96 (rows 65:95 are stale psum; cols unused)
                    o_t = o_pool.tile([96, SB], f32r, tag="ot")
                    nc.scalar.copy(o_t[0:D, :], ps_o[0:D, :])
                    nc.scalar.copy(o_t[D:96, :], ps_o[D:96, :])
                    for st in range(4):
                        ps_t = ps_sm.tile([128, 128], f32r, tag="sm")
                        nc.tensor.transpose(
                            ps_t[:, 0:96],
                            o_t[:, st * 128:(st + 1) * 128],
                            ident[0:96, 0:96],
                        )
                        recip = o_pool.tile([128, 1], f32, tag="recip")
                        nc.vector.reciprocal(recip, ps_t[:, D:D + 1])
                        outf = o_pool.tile([128, D], f32, tag="outf")
                        nc.vector.tensor_scalar_mul(outf, ps_t[:, 0:D], recip)
                        row = (jb * 4 + st) * 128
                        nc.sync.dma_start(out=OUT[row:row + 128, :], in_=outf)

    nc.compile()
    return nc


_NC = None


def kernel(x, mask, Wq, bq, Wk, bk, Wv, bv):
    global _NC
    if _NC is None:
        _NC = build_nc()
    from concourse.bass_utils import run_bass_kernel_spmd

    x = np.ascontiguousarray(np.asarray(x, dtype=np.float32))
    mask = np.ascontiguousarray(np.asarray(mask, dtype=np.int32))
    shared = {
        "Wq": np.asarray(Wq, np.float32), "bq": np.asarray(bq, np.float32),
        "Wk": np.asarray(Wk, np.float32), "bk": np.asarray(bk, np.float32),
        "Wv": np.asarray(Wv, np.float32), "bv": np.asarray(bv, np.float32),
    }
    in_maps = [dict(x_b=x[c], mask_b=mask[c], **shared) for c in range(B)]
    res = run_bass_kernel_spmd(_NC, in_maps, core_ids=list(range(B)))
    return np.stack([res.results[c]["out_b"] for c in range(B)], axis=0)


# revision 14
# speedup vs baseline: 1294.8187x; 1.4114x over previous
"""Single-head attention (B=8, S=2048, H=1024, D=64) on 8 TRN2 NeuronCores.

Sharding: data-parallel over the batch dim — core b computes batch element b.

Per-core dataflow (everything kept transposed so the PE contraction dim is
always the partition dim and softmax denominators come out of the attn@v
matmul itself):
  xT[h, s]   = PE-transpose of x tiles                 (f32r)
  qT/kT/vT   = W.T @ xT  (+bias fused on ScalarE)      (f32r, [64, S])
  v_aug      = [v | 1] in [t, d] layout                (f32r, [S, 65])
  scoresT    = kT_tile.T @ qT                          (PSUM, [t=128, s=512])
  attnT      = exp(scoresT/8 + mask_bias)  on ScalarE  (f32r)
  outT_aug   = sum_t v_aug.T @ attnT  (PSUM accum)     ([65, 512]; row 64 = denom)
  out        = transpose(outT_aug) , then out[:, :64] * 1/out[:, 64]
"""

import sys

sys.path.insert(0, "/opt/trn_rl_repo")

import numpy as np

B, S, H, D = 8, 2048, 1024, 64
SB = 512          # s-block (streaming block of queries)
NBLK = S // SB    # 4
NT = S // 128     # 16 t-tiles (and s-tiles)
HC = H // 128     # 8 h-chunks
MASK_BIG = 30000.0


def build_nc(repeats=1):
    import concourse.bacc as bacc
    import concourse.mybir as mybir
    import concourse.tile as tile
    from concourse.masks import make_identity

    dt = mybir.dt
    f32, f32r, i32 = dt.float32, dt.float32r, dt.int32
    AF = mybir.ActivationFunctionType

    nc = bacc.Bacc("TRN2", target_bir_lowering=False, debug=False, num_devices=8)

    X = nc.dram_tensor("x_b", [S, H], f32, kind="ExternalInput")
    MASK = nc.dram_tensor("mask_b", [S], i32, kind="ExternalInput")
    WQ = nc.dram_tensor("Wq", [H, D], f32, kind="ExternalInput")
    BQ = nc.dram_tensor("bq", [D], f32, kind="ExternalInput")
    WK = nc.dram_tensor("Wk", [H, D], f32, kind="ExternalInput")
    BK = nc.dram_tensor("bk", [D], f32, kind="ExternalInput")
    WV = nc.dram_tensor("Wv", [H, D], f32, kind="ExternalInput")
    BV = nc.dram_tensor("bv", [D], f32, kind="ExternalInput")
    OUT = nc.dram_tensor("out_b", [S, D], f32, kind="ExternalOutput")

    with tile.TileContext(nc) as tc:
        with (
            tc.tile_pool(name="const", bufs=1) as cpool,
            tc.tile_pool(name="xs", bufs=5) as xs_pool,
            tc.tile_pool(name="xt", bufs=1) as xt_pool,
            tc.tile_pool(name="qkv", bufs=1) as qkv_pool,
            tc.tile_pool(name="attn", bufs=2) as at_pool,
            tc.tile_pool(name="outs", bufs=2) as o_pool,
            tc.tile_pool(name="ps_big", bufs=4, space="PSUM") as ps_big,
            tc.tile_pool(name="ps_acc", bufs=2, space="PSUM") as ps_acc,
            tc.tile_pool(name="ps_sm", bufs=2, space="PSUM") as ps_sm,
        ):
            # ---- constants ----
            ident_f = cpool.tile([128, 128], f32)
            make_identity(nc, ident_f)
            ident = cpool.tile([128, 128], f32r)
            nc.vector.tensor_copy(ident, ident_f)

            # fused [Wq | Wk] stationary: one projection matmul makes q and k
            wqk = cpool.tile([128, HC, 2 * D], f32r)
            wv = cpool.tile([128, HC, D], f32r)
            nc.gpsimd.dma_start(out=wqk[:, :, 0:D], in_=WQ.ap().rearrange("(c p) m -> p c m", p=128))
            nc.gpsimd.dma_start(out=wqk[:, :, D:2 * D], in_=WK.ap().rearrange("(c p) m -> p c m", p=128))
            nc.gpsimd.dma_start(out=wv, in_=WV.ap().rearrange("(c p) m -> p c m", p=128))

            bias_qk = cpool.tile([128, 1], f32)
            bias_v = cpool.tile([D, 1], f32)
            nc.gpsimd.dma_start(out=bias_qk[0:D, :], in_=BQ.ap().rearrange("(p o) -> p o", o=1))
            nc.gpsimd.dma_start(out=bias_qk[D:2 * D, :], in_=BK.ap().rearrange("(p o) -> p o", o=1))
            nc.gpsimd.dma_start(out=bias_v, in_=BV.ap().rearrange("(p o) -> p o", o=1))

            # mask -> per-t-chunk additive bias column: (mask - 1) * MASK_BIG
            mask_i = cpool.tile([128, NT], i32)
            nc.gpsimd.dma_start(out=mask_i, in_=MASK.ap().rearrange("(c p) -> p c", p=128))
            mask_f = cpool.tile([128, NT], f32)
            nc.vector.tensor_copy(mask_f, mask_i)
            ones_col = cpool.tile([128, 1], f32)
            nc.vector.memset(ones_col, 1.0)
            zeros_pad = cpool.tile([128, 96 - D - 1], f32)
            nc.vector.memset(zeros_pad, 0.0)
            mask_bias = cpool.tile([128, NT], f32)
            nc.vector.tensor_scalar(
                out=mask_bias, in0=mask_f,
                scalar1=1.0, scalar2=MASK_BIG,
                op0=mybir.AluOpType.subtract, op1=mybir.AluOpType.mult,
            )

            for _rep in range(repeats):
                # ---- stage A: load x, transpose to xT ----
                xt = xt_pool.tile([128, HC, S], f32r, tag="xt")
                for jb in range(NBLK):
                    xs_tiles = []
                    for i4 in range(4):
                        xs = xs_pool.tile([128, H], f32r, tag="xs")
                        st = jb * 4 + i4
                        nc.gpsimd.dma_start(out=xs, in_=X[st * 128:(st + 1) * 128, :])
                        xs_tiles.append(xs)
                    for c in range(HC):
                        ps = ps_big.tile([128, SB], f32r, tag="big")
                        for i4 in range(4):
                            nc.tensor.transpose(
                                ps[:, i4 * 128:(i4 + 1) * 128],
                                xs_tiles[i4][:, c * 128:(c + 1) * 128],
                                ident,
                            )
                        dst = xt[:, c, jb * SB:(jb + 1) * SB]
                        if c % 2 == 0:
                            nc.vector.tensor_copy(dst, ps)
                        else:
                            nc.scalar.copy(dst, ps)

                # ---- stage B: projections (+bias), build v_aug ----
                # stack_qk = [qT ; kT] (q rows 0:64, k rows 64:128) straight
                # from the fused projection; stack_kq is the swapped copy so
                # both PE row-halves see both tensors (row-tiled scores).
                stack_qk = qkv_pool.tile([128, S], f32r, tag="stack_qk")
                stack_kq = qkv_pool.tile([128, S], f32r, tag="stack_kq")
                vT = qkv_pool.tile([D, S], f32r, tag="vT")
                v_aug = qkv_pool.tile([128, NT, 96], f32r, tag="v_aug")
                for j in range(NBLK):
                    sl = slice(j * SB, (j + 1) * SB)
                    ps = ps_big.tile([128, SB], f32, tag="big")
                    for c in range(HC):
                        nc.tensor.matmul(
                            ps, wqk[:, c, :], xt[:, c, sl],
                            start=(c == 0), stop=(c == HC - 1),
                        )
                    nc.scalar.add(stack_qk[:, sl], ps, bias_qk)
                    nc.sync.dma_start(out=stack_kq[0:D, sl], in_=stack_qk[D:2 * D, sl])
                    nc.sync.dma_start(out=stack_kq[D:2 * D, sl], in_=stack_qk[0:D, sl])

                    ps_v = ps_big.tile([128, SB], f32, tag="big")
                    for c in range(HC):
                        nc.tensor.matmul(
                            ps_v[0:D, :], wv[:, c, :], xt[:, c, sl],
                            start=(c == 0), stop=(c == HC - 1),
                        )
                    nc.scalar.add(vT[:, sl], ps_v[0:D, :], bias_v)
                    # v_aug tiles for the 4 s-tiles of this block
                    for i4 in range(4):
                        i = j * 4 + i4
                        ps_t = ps_sm.tile([128, 128], f32r, tag="sm")
                        nc.tensor.transpose(
                            ps_t[:, 0:D], vT[:, i * 128:(i + 1) * 128], ident[0:D, 0:D]
                        )
                        nc.vector.tensor_copy(v_aug[:, i, 0:D], ps_t[:, 0:D])
                        nc.vector.tensor_copy(v_aug[:, i, D:D + 1], ones_col)
                        nc.vector.tensor_copy(v_aug[:, i, D + 1:96], zeros_pad)

                # ---- stage C: attention per s-block ----
                # pass 1: scores on a row-split PE (64x128 tiling, T0 and T8
                # run concurrently, each contracting its own K=64 operands);
                # pass 2: attn@v untiled (K=128 uses the full array).
                for jb in range(NBLK):
                    sl = slice(jb * SB, (jb + 1) * SB)
                    at = at_pool.tile([128, NT, SB], f32r, tag="at")
                    for ih in range(NT // 2):
                        for half in (0, 1):
                            i = ih + half * (NT // 2)
                            tsl = slice(i * 128, (i + 1) * 128)
                            ps = ps_big.tile([128, SB], f32, tag="big")
                            if half == 0:
                                nc.tensor.matmul(
                                    ps, stack_kq[0:D, tsl], stack_qk[0:D, sl],
                                    start=True, stop=True, tile_position=(0, 0),
                                )
                            else:
                                nc.tensor.matmul(
                                    ps, stack_qk[D:2 * D, tsl], stack_kq[D:2 * D, sl],
                                    start=True, stop=True, tile_position=(64, 0),
                                )
                            nc.scalar.activation(
                                out=at[:, i, :], in_=ps, func=AF.Exp,
                                bias=mask_bias[:, i:i + 1], scale=0.125,
                            )
                    ps_o = ps_acc.tile([128, SB], f32, tag="acc")
                    for i in range(NT):
                        nc.tensor.matmul(
                            ps_o[0:96, :], v_aug[:, i, :], at[:, i, :],
                            start=(i == 0), stop=(i == NT - 1),
                        )
                    # transpose partition count must be a multiple of 32 -> pad
                    # 65 rows to 96 (rows 65:95 are stale psum; cols unused)
                    o_t = o_pool.tile([96, SB], f32r, tag="ot")
                    nc.scalar.copy(o_t, ps_o[0:96, :])
                    for st in range(4):
                        ps_t = ps_sm.tile([128, 128], f32r, tag="sm")
                        nc.tensor.transpose(
                            ps_t[:, 0:96],
                            o_t[:, st * 128:(st + 1) * 128],
                            ident[0:96, 0:96],
                        )
                        recip = o_pool.tile([128, 1], f32, tag="recip")
                        nc.vector.reciprocal(recip, ps_t[:, D:D + 1])
                        outf = o_pool.tile([128, D], f32, tag="outf")
                        nc.vector.tensor_scalar_mul(outf, ps_t[:, 0:D], recip)
                        row = (jb * 4 + st) * 128
                        nc.sync.dma_start(out=OUT[row:row + 128, :], in_=outf)

    nc.compile()
    return nc


_NC = None


def kernel(x, mask, Wq, bq, Wk, bk, Wv, bv):
    global _NC
    if _NC is None:
        _NC = build_nc()
    from concourse.bass_utils import run_bass_kernel_spmd

    x = np.ascontiguousarray(np.asarray(x, dtype=np.float32))
    mask = np.ascontiguousarray(np.asarray(mask, dtype=np.int32))
    shared = {
        "Wq": np.asarray(Wq, np.float32), "bq": np.asarray(bq, np.float32),
        "Wk": np.asarray(Wk, np.float32), "bk": np.asarray(bk, np.float32),
        "Wv": np.asarray(Wv, np.float32), "bv": np.asarray(bv, np.float32),
    }
    in_maps = [dict(x_b=x[c], mask_b=mask[c], **shared) for c in range(B)]
    res = run_bass_kernel_spmd(_NC, in_maps, core_ids=list(range(B)))
    return np.stack([res.results[c]["out_b"] for c in range(B)], axis=0)
